# revision 1
# baseline (speedup 1.0000x reference)
"""Trainium2 Bass kernel for nn_LlamaAttention_cam (sparse_attention).

Sharding: 16 heads across 8 cores (2 heads/core), both batches per core.
Q/K/V projections column-parallel over heads; o_proj row-parallel (per-core
partial outputs summed on host). The CaM merge is a rank-1 correction
(s_tail outer v_e) applied on host from tiny device-side statistics.

Self-contained: hardcodes all shapes; takes full inputs, returns full output.
"""

import math
import os

import numpy as np
import ml_dtypes

B, T, HID, H = 2, 2048, 2048, 16
D = 128
NCORES = 8
HL = H // NCORES  # heads per core = 2
BT = B * T  # 4096
NF = HID // 128  # 16 f-tiles
SCALE = 1.0 / math.sqrt(D)
RB = int(0.25 * T)  # 512 recent budget
WS = T - RB  # 1536
EVICT = WS - 1  # 1535

# jax.random.uniform(jax.random.key(42), (2,16), float32); bernoulli(key,p) == u < p
U_CONST = np.array(
    [[0.59400654, 0.43801308, 0.6285691, 0.00791204, 0.27834702,
      0.7976179, 0.8521497, 0.9625306, 0.67656493, 0.11104441,
      0.4959929, 0.7311437, 0.18970704, 0.1544199, 0.03802836,
      0.33559263],
     [0.92825687, 0.6123972, 0.49262476, 0.733806, 0.18920851,
      0.15386605, 0.037136197, 0.32930005, 0.9372028, 0.5957513,
      0.4615929, 0.6695677, 0.07019377, 0.39408123, 0.55786455,
      0.35412872]], dtype=np.float32)

BF16 = ml_dtypes.bfloat16

_NC_CACHE = {}


def build_nc():
    import concourse.bacc as bacc
    import concourse.mybir as mybir
    import concourse.tile as tile

    f32 = mybir.dt.float32
    bf16 = mybir.dt.bfloat16
    EXP = mybir.ActivationFunctionType.Exp

    nc = bacc.Bacc("TRN2", target_bir_lowering=False, debug=False)
    env = os.environ
    B_QK = int(env.get("BK_QKPS", "4")); B_VPS = int(env.get("BK_VPS", "2"))
    B_HSP = int(env.get("BK_HSP", "2")); B_SPS = int(env.get("BK_SPS", "2"))
    B_OPS = int(env.get("BK_OPS", "3")); B_SM = int(env.get("BK_SM", "1"))
    WIDE = env.get("BK_WIDE", "1") == "1"
    B_ROPE = int(env.get("BK_ROPE", "3"))

    hsT = nc.dram_tensor("hsT", [HID, BT], bf16, kind="ExternalInput")
    wq = nc.dram_tensor("wq", [HID, 256], bf16, kind="ExternalInput")
    wk = nc.dram_tensor("wk", [HID, 256], bf16, kind="ExternalInput")
    wv = nc.dram_tensor("wv", [HID, 256], bf16, kind="ExternalInput")
    wo = nc.dram_tensor("wo", [256, HID], bf16, kind="ExternalInput")
    cosd = nc.dram_tensor("cosT", [128, T], f32, kind="ExternalInput")
    sind = nc.dram_tensor("sinT", [128, T], f32, kind="ExternalInput")
    maskd = nc.dram_tensor("masks", [128, 2048], bf16, kind="ExternalInput")

    outT = nc.dram_tensor("outT", [HID, BT], bf16, kind="ExternalOutput")
    abard = nc.dram_tensor("abar", [4, 128, 16], f32, kind="ExternalOutput")
    sumsd = nc.dram_tensor("sums", [4, 2, T], f32, kind="ExternalOutput")

    with tile.TileContext(nc) as tc:
        with (
            tc.tile_pool(name="singles", bufs=1) as singles,
            tc.tile_pool(name="res", bufs=1) as res,
            tc.tile_pool(name="stats", bufs=1) as stats,
        ):
            # --- constant loads ---
            wq_sb = singles.tile([128, NF, 256], bf16, tag="wq")
            wk_sb = singles.tile([128, NF, 256], bf16, tag="wk")
            wv_sb = singles.tile([128, NF, 256], bf16, tag="wv")
            for dst, src in ((wq_sb, wq), (wk_sb, wk), (wv_sb, wv)):
                nc.sync.dma_start(
                    out=dst, in_=src.rearrange("(nf p) d -> p nf d", p=128)
                )
            wo_sb = singles.tile([128, 2, HID], bf16, tag="wo")
            nc.sync.dma_start(
                out=wo_sb, in_=wo.rearrange("(kt p) f -> p kt f", p=128)
            )
            cos_sb = singles.tile([128, T], f32, tag="cos")
            sin_sb = singles.tile([128, T], f32, tag="sin")
            nc.sync.dma_start(out=cos_sb, in_=cosd[:, :])
            nc.sync.dma_start(out=sin_sb, in_=sind[:, :])
            mask_sb = singles.tile([128, 4, 512], bf16, tag="mask")
            nc.sync.dma_start(
                out=mask_sb, in_=maskd.rearrange("p (v t) -> p v t", v=4)
            )
            ones_a = singles.tile([128, 2], bf16, tag="onesa")  # [1, 0]
            ones_b = singles.tile([128, 2], bf16, tag="onesb")  # [1, 1]
            nc.vector.memset(ones_a[:, 0:1], 1.0)
            nc.vector.memset(ones_a[:, 1:2], 0.0)
            nc.vector.memset(ones_b, 1.0)

            # --- residents ---
            qt = [res.tile([128, BT], bf16, tag=f"qt{h}", name=f"qt{h}") for h in range(HL)]
            kt = [res.tile([128, BT], bf16, tag=f"kt{h}", name=f"kt{h}") for h in range(HL)]
            vres = res.tile([128, 32, 256], bf16, tag="vres")
            ot = [res.tile([128, T], bf16, tag=f"ot{p}", name=f"ot{p}") for p in range(4)]
            abar_raw = [stats.tile([128, 16], f32, tag=f"ab{p}", name=f"ab{p}") for p in range(4)]

            import os as _os
            _ph = _os.environ.get("BK_PHASES", "123")
            # ================= Phase 1: QKV projections + RoPE =================
            with (
                tc.tile_pool(name="hsp", bufs=B_HSP) as hsp,
                tc.tile_pool(name="rope", bufs=B_ROPE) as rope,
                tc.tile_pool(name="qkps", bufs=B_QK, space="PSUM") as qkps,
                tc.tile_pool(name="vps", bufs=B_VPS, space="PSUM") as vps,
            ):
                PAIR = os.environ.get("BK_PAIR", "1") == "1"
                for c0 in (range(0, 8, 2) if "1" in _ph else []):
                    hs_pair = []
                    for c in (c0, c0 + 1):
                        cs = slice(c * 512, (c + 1) * 512)
                        hs_t = hsp.tile([128, NF, 512], bf16, tag="hs")
                        nc.sync.dma_start(
                            out=hs_t,
                            in_=hsT[:, cs].rearrange("(nf p) t -> p nf t", p=128),
                        )
                        hs_pair.append(hs_t)
                    for h in range(HL):
                        for w_sb, dest in ((wq_sb, qt[h]), (wk_sb, kt[h])):
                            pss = [qkps.tile([128, 512], f32, tag="qk",
                                              name=f"qkp{i}")
                                   for i in range(2)]
                            for f in range(NF):
                                for i in range(2):
                                    nc.tensor.matmul(
                                        pss[i],
                                        lhsT=w_sb[:, f, h * 128:(h + 1) * 128],
                                        rhs=hs_pair[i][:, f, :],
                                        start=(f == 0),
                                        stop=(f == NF - 1),
                                    )
                            for i, c in enumerate((c0, c0 + 1)):
                                cs = slice(c * 512, (c + 1) * 512)
                                tl = slice((c % 4) * 512, (c % 4) * 512 + 512)
                                ps = pss[i]
                                qf = rope.tile([128, 512], f32, tag="qf")
                                nc.scalar.copy(qf, ps)
                                rot = rope.tile([128, 512], f32, tag="rot")
                                nc.gpsimd.dma_start(out=rot[0:64, :], in_=qf[64:128, :])
                                nc.gpsimd.dma_start(out=rot[64:128, :], in_=qf[0:64, :])
                                t1 = rope.tile([128, 512], f32, tag="t1")
                                nc.vector.tensor_mul(t1, rot, sin_sb[:, tl])
                                t2 = rope.tile([128, 512], f32, tag="t2")
                                nc.vector.tensor_mul(t2, qf, cos_sb[:, tl])
                                nc.vector.tensor_add(dest[:, cs], t1, t2)
                    for i, c in enumerate((c0, c0 + 1)):
                        for s in range(4):
                            vp = vps.tile([128, 256], f32, tag="v")
                            for f in range(NF):
                                nc.tensor.matmul(
                                    vp,
                                    lhsT=hs_pair[i][:, f, s * 128:(s + 1) * 128],
                                    rhs=wv_sb[:, f, :],
                                    start=(f == 0),
                                    stop=(f == NF - 1),
                                )
                            nc.scalar.copy(vres[:, c * 4 + s, :], vp)

            # ========== Phase 2+3: attention + interleaved o_proj ==========
            with (
                tc.tile_pool(name="sps", bufs=B_SPS, space="PSUM") as sps,
                tc.tile_pool(name="ops", bufs=B_OPS, space="PSUM") as ops,
                tc.tile_pool(name="smps", bufs=B_SM, space="PSUM") as smps,
                tc.tile_pool(name="pt", bufs=4) as ptp,
                tc.tile_pool(name="att_sm", bufs=4) as atsm,
                tc.tile_pool(name="ob", bufs=2) as obp,
            ):
                def attn_chunk(p, c):
                    b, h = p // 2, p % 2
                    jmax = 4 * (c + 1)
                    cl = slice(c * 512, (c + 1) * 512)
                    qtb, ktb = qt[h], kt[h]
                    o_ps = ops.tile([128, 512], f32, tag="o")
                    sm_ps = smps.tile([2, 512], f32, tag="sm")

                    def emit_s(j):
                        sp = sps.tile([128, 512], f32, tag="s")
                        nc.tensor.matmul(
                            sp,
                            lhsT=ktb[:, b * T + j * 128: b * T + (j + 1) * 128],
                            rhs=qtb[:, b * T + c * 512: b * T + (c + 1) * 512],
                            start=True,
                            stop=True,
                        )
                        return sp

                    def emit_epv(j, sp):
                        pt_t = ptp.tile([128, 512], bf16, tag="p")
                        nc.scalar.activation(pt_t, sp, EXP, scale=SCALE)
                        if j >= 4 * c:
                            nc.vector.tensor_mul(
                                pt_t, pt_t, mask_sb[:, j - 4 * c, :]
                            )
                        if c == 3:
                            nc.vector.tensor_copy(
                                abar_raw[p][:, j:j + 1], sp[:, 511:512]
                            )
                        nc.tensor.matmul(
                            o_ps,
                            lhsT=vres[:, b * 16 + j, h * 128:(h + 1) * 128],
                            rhs=pt_t,
                            start=(j == 0),
                            stop=(j == jmax - 1),
                        )
                        nc.tensor.matmul(
                            sm_ps,
                            lhsT=(ones_b if j >= 12 else ones_a),
                            rhs=pt_t,
                            start=(j == 0),
                            stop=(j == jmax - 1),
                        )

                    if not WIDE:
                        sp_q = [emit_s(0)]
                        if jmax > 1:
                            sp_q.append(emit_s(1))
                        for j in range(jmax):
                            if j + 2 < jmax:
                                sp_q.append(emit_s(j + 2))
                            emit_epv(j, sp_q[j])
                    else:
                        def emit_s2(m):
                            sp = sps.tile([128, 1024], f32, tag="s")
                            for i, j in ((0, 2 * m), (1, 2 * m + 1)):
                                nc.tensor.matmul(
                                    sp[:, i * 512:(i + 1) * 512],
                                    lhsT=ktb[:, b * T + j * 128: b * T + (j + 1) * 128],
                                    rhs=qtb[:, b * T + c * 512: b * T + (c + 1) * 512],
                                    start=True, stop=True,
                                )
                            return sp

                        def emit_epv2(m, sp):
                            pt_t = ptp.tile([128, 1024], bf16, tag="p")
                            nc.scalar.activation(pt_t, sp, EXP, scale=SCALE)
                            for i, j in ((0, 2 * m), (1, 2 * m + 1)):
                                pth = pt_t[:, i * 512:(i + 1) * 512]
                                if j >= 4 * c:
                                    nc.vector.tensor_mul(pth, pth, mask_sb[:, j - 4 * c, :])
                                if c == 3:
                                    nc.vector.tensor_copy(
                                        abar_raw[p][:, j:j + 1],
                                        sp[:, i * 512 + 511: i * 512 + 512],
                                    )
                                nc.tensor.matmul(
                                    o_ps,
                                    lhsT=vres[:, b * 16 + j, h * 128:(h + 1) * 128],
                                    rhs=pth,
                                    start=(j == 0), stop=(j == jmax - 1),
                                )
                                nc.tensor.matmul(
                                    sm_ps,
                                    lhsT=(ones_b if j >= 12 else ones_a),
                                    rhs=pth,
                                    start=(j == 0), stop=(j == jmax - 1),
                                )

                        mmax = jmax // 2
                        sq = [emit_s2(0)]
                        for m in range(mmax):
                            if m + 1 < mmax:
                                sq.append(emit_s2(m + 1))
                            emit_epv2(m, sq[m])

                    rec = atsm.tile([1, 512], f32, tag="rec")
                    nc.vector.reciprocal(rec, sm_ps[0:1, :])
                    bc = atsm.tile([128, 512], f32, tag="bc")
                    nc.gpsimd.partition_broadcast(bc, rec)
                    nc.vector.tensor_mul(ot[p][:, cl], o_ps, bc)
                    sm_sb = atsm.tile([2, 512], f32, tag="smsb")
                    nc.vector.tensor_copy(sm_sb, sm_ps)
                    nc.sync.dma_start(out=sumsd[p, :, cl], in_=sm_sb)

                def oproj_chunk(b, c):
                    tl = slice(c * 512, (c + 1) * 512)
                    cg = slice((b * 4 + c) * 512, (b * 4 + c + 1) * 512)
                    ob = obp.tile([128, 16, 512], bf16, tag="ob")
                    for fo in range(16):
                        fs = slice(fo * 128, (fo + 1) * 128)
                        pp = ops.tile([128, 512], f32, tag="o")
                        nc.tensor.matmul(
                            pp, lhsT=wo_sb[:, 0, fs], rhs=ot[b * 2 + 0][:, tl],
                            start=True, stop=False,
                        )
                        nc.tensor.matmul(
                            pp, lhsT=wo_sb[:, 1, fs], rhs=ot[b * 2 + 1][:, tl],
                            start=False, stop=True,
                        )
                        if fo % 2 == 0:
                            nc.vector.tensor_copy(ob[:, fo, :], pp)
                        else:
                            nc.scalar.copy(ob[:, fo, :], pp)
                    nc.sync.dma_start(
                        out=outT[:, cg].rearrange("(nf p) t -> p nf t", p=128),
                        in_=ob,
                    )

                if "2" in _ph:
                    for b in range(B):
                        for c in range(4):
                            attn_chunk(b * 2 + 0, c)
                            attn_chunk(b * 2 + 1, c)
                            if "3" in _ph:
                                oproj_chunk(b, c)
                        for hl in range(HL):
                            p = b * 2 + hl
                            ab_exp = atsm.tile([128, 16], f32, tag="abe")
                            nc.scalar.activation(
                                ab_exp, abar_raw[p], EXP, scale=SCALE
                            )
                            nc.sync.dma_start(out=abard[p], in_=ab_exp)

    nc.compile()
    return nc


def _get_nc():
    if "nc" not in _NC_CACHE:
        _NC_CACHE["nc"] = build_nc()
    return _NC_CACHE["nc"]


def _host_inputs(hidden_states, q_w, k_w, v_w, o_w):
    """Per-core input dicts."""
    hsT = np.ascontiguousarray(
        hidden_states.reshape(BT, HID).T).astype(BF16)
    inv = 10000.0 ** (-np.arange(64, dtype=np.float64) / 64.0)
    t = np.arange(T, dtype=np.float64)
    fr = t[None, :] * inv[:, None]  # [64, T]
    cosT = np.cos(np.concatenate([fr, fr], 0)).astype(np.float32)
    sinT = np.sin(np.concatenate([fr, fr], 0)).astype(np.float32)
    sinT[:64] *= -1.0  # sign-baked for swap-halves rotate
    masks = np.zeros((128, 4, 512), dtype=np.float32)
    kk = np.arange(128)[:, None]
    tt = np.arange(512)[None, :]
    for v in range(4):
        masks[:, v, :] = (tt >= 128 * v + kk).astype(np.float32)
    masks = masks.reshape(128, 2048).astype(BF16)

    in_maps = []
    for core in range(NCORES):
        rs = slice(core * 256, (core + 1) * 256)
        in_maps.append({
            "hsT": hsT,
            "wq": np.ascontiguousarray(q_w[rs, :].T).astype(BF16),
            "wk": np.ascontiguousarray(k_w[rs, :].T).astype(BF16),
            "wv": np.ascontiguousarray(v_w[rs, :].T).astype(BF16),
            "wo": np.ascontiguousarray(o_w[:, rs].T).astype(BF16),
            "cosT": cosT,
            "sinT": sinT,
            "masks": masks,
        })
    return in_maps


def _epilogue(out, results, hidden_states, v_w, o_w):
    """Add the CaM rank-1 correction per (b, h) on host."""
    for core in range(NCORES):
        r = results[core]
        for p in range(4):
            b, hl = p // 2, p % 2
            h = core * HL + hl
            rowsum = r["sums"][p][0]  # [T] unnormalized exp row sums
            tails = r["sums"][p][1]
            a_exp = np.asarray(r["abar"][p], np.float64).T.reshape(2048)
            a_bar = a_exp / max(float(rowsum[T - 1]), 1e-30)
            avg_w = max(float(np.mean(a_bar[WS:])), 1e-6)
            prob = float(np.clip(a_bar[EVICT] / avg_w, 0.0, 1.0))
            prob = float(np.nan_to_num(prob, nan=0.0, posinf=1.0, neginf=0.0))
            m = 1.0 if U_CONST[b, h] < prob else 0.0
            if m == 0.0:
                continue
            # exact v_e from fp32 inputs
            v_row = hidden_states[b, EVICT, :] @ v_w[h * D:(h + 1) * D, :].T
            v_e = v_row * (m / RB)  # [D]
            w_e = o_w[:, h * D:(h + 1) * D] @ v_e  # [HID]
            s_tail = (tails / np.maximum(rowsum, 1e-30)).astype(np.float32)
            out[b] += np.outer(s_tail, w_e).astype(np.float32)
    return out


def kernel(hidden_states, attention_mask, q_w, k_w, v_w, o_w):
    from concourse.bass_utils import run_bass_kernel_spmd

    nc = _get_nc()
    in_maps = _host_inputs(hidden_states, q_w, k_w, v_w, o_w)
    trace = bool(int(os.environ.get("BK_TRACE", "0")))
    res = run_bass_kernel_spmd(
        nc, in_maps, core_ids=list(range(NCORES)), trace=trace,
    )
    if trace and res.exec_time_ns is not None:
        print(f"HW exec time: {res.exec_time_ns} ns")
        _NC_CACHE["last_exec_ns"] = res.exec_time_ns
        _NC_CACHE["last_trace"] = res.instructions_and_trace
    results = res.results

    acc = np.zeros((HID, BT), dtype=np.float32)
    for core in range(NCORES):
        acc += np.asarray(results[core]["outT"], np.float32)
    out = np.ascontiguousarray(acc.T).reshape(B, T, HID)
    out = _epilogue(out, results, hidden_states, v_w, o_w)
    return out.astype(np.float32)



# revision 15
# speedup vs baseline: 1.1323x; 1.1323x over previous
"""Trainium2 Bass kernel for nn_LlamaAttention_cam (sparse_attention).

Sharding: 16 heads across 8 cores (2 heads/core), both batches per core.
Q/K/V projections column-parallel over heads; o_proj row-parallel (per-core
partial outputs summed on host). The CaM merge is a rank-1 correction
(s_tail outer v_e) applied on host from tiny device-side statistics; the
bernoulli decision is recomputed exactly on host (needs only last-row
scores for the 513 tail keys — the softmax denominator cancels).

Projections run as 3-term residual fp8 DoubleRow matmuls:
  64*W*hs = W64(x)hs_hi + W8(x)hs_lo8 + Wres8(x)hs_d8
with W64=fp8(64W), W8=fp8(8W), Wres8=fp8(8(64W-W64)), hs_hi=fp8(hs),
hs_lo8=fp8(8(hs-hs_hi)), hs_d8=fp8(hs/8). Each term pairs two 128-deep
k-subtiles per DoubleRow instruction, so the projection costs 0.75x of
bf16 while retaining ~bf16 accuracy. Scores / P@V / row-sum / o_proj stay
bf16 (fp8 there fails the accuracy gate: softmax output is a near-zero-
mean average, so per-element fp8 noise does not average down).

Self-contained: hardcodes all shapes; takes full inputs, returns full output.
"""

import math
import os

import numpy as np
import ml_dtypes

B, T, HID, H = 2, 2048, 2048, 16
D = 128
NCORES = 8
HL = H // NCORES  # heads per core = 2
BT = B * T  # 4096
NF = HID // 128  # 16 f-tiles
SCALE = 1.0 / math.sqrt(D)
RB = int(0.25 * T)  # 512 recent budget
WS = T - RB  # 1536
EVICT = WS - 1  # 1535
WSCALE = 64.0  # fp8 weight pre-scale

# jax.random.uniform(jax.random.key(42), (2,16), float32); bernoulli(key,p) == u < p
U_CONST = np.array(
    [[0.59400654, 0.43801308, 0.6285691, 0.00791204, 0.27834702,
      0.7976179, 0.8521497, 0.9625306, 0.67656493, 0.11104441,
      0.4959929, 0.7311437, 0.18970704, 0.1544199, 0.03802836,
      0.33559263],
     [0.92825687, 0.6123972, 0.49262476, 0.733806, 0.18920851,
      0.15386605, 0.037136197, 0.32930005, 0.9372028, 0.5957513,
      0.4615929, 0.6695677, 0.07019377, 0.39408123, 0.55786455,
      0.35412872]], dtype=np.float32)

BF16 = ml_dtypes.bfloat16
E4 = ml_dtypes.float8_e4m3

_NC_CACHE = {}


def _flag(name, default):
    return os.environ.get(name, default) == "1"


QKR = _flag("BK_QKR", "1")   # Q/K projections residual-fp8 DoubleRow
VR = _flag("BK_VR", "1")     # V projection residual-fp8 DoubleRow
LAG = _flag("BK_LAG", "1")   # o_proj emission lags attn by one chunk


def build_nc():
    import concourse.bacc as bacc
    import concourse.mybir as mybir
    import concourse.tile as tile

    f32 = mybir.dt.float32
    bf16 = mybir.dt.bfloat16
    fp8 = mybir.dt.float8e4
    EXP = mybir.ActivationFunctionType.Exp
    COPY = mybir.ActivationFunctionType.Copy
    DR = mybir.MatmulPerfMode.DoubleRow

    nc = bacc.Bacc("TRN2", target_bir_lowering=False, debug=False)
    env = os.environ
    B_QK = int(env.get("BK_QKPS", "4")); B_VPS = int(env.get("BK_VPS", "2"))
    B_HSP = int(env.get("BK_HSP", "2")); B_SPS = int(env.get("BK_SPS", "2"))
    B_OPS = int(env.get("BK_OPS", "3")); B_SM = int(env.get("BK_SM", "1"))
    B_ROPE = int(env.get("BK_ROPE", "3"))

    any_r = QKR or VR
    hs3 = nc.dram_tensor("hs3", [3, HID, BT], fp8, kind="ExternalInput")
    if not (QKR and VR):
        hsT16 = nc.dram_tensor("hsT16", [HID, BT], bf16, kind="ExternalInput")
    wq = nc.dram_tensor("wq", [3, HID, 256], fp8 if QKR else bf16,
                        kind="ExternalInput")
    wk = nc.dram_tensor("wk", [3, HID, 256], fp8 if QKR else bf16,
                        kind="ExternalInput")
    wv = nc.dram_tensor("wv", [3, HID, 256], fp8 if VR else bf16,
                        kind="ExternalInput")
    wo = nc.dram_tensor("wo", [256, HID], bf16, kind="ExternalInput")
    cosd = nc.dram_tensor("cosT", [128, T], bf16, kind="ExternalInput")
    sind = nc.dram_tensor("sinT", [128, T], bf16, kind="ExternalInput")
    maskd = nc.dram_tensor("masks", [128, 2048], bf16, kind="ExternalInput")

    outT = nc.dram_tensor("outT", [HID, BT], bf16, kind="ExternalOutput")
    sumsd = nc.dram_tensor("sums", [4, 2, T], f32, kind="ExternalOutput")

    with tile.TileContext(nc) as tc:
        with (
            tc.tile_pool(name="singles", bufs=1) as singles,
            tc.tile_pool(name="res", bufs=1) as res,
        ):
            # --- constant loads: wq/wk on SP queue; the rest on Act queue ---
            wq_sb = singles.tile([128, 3, NF, 256], fp8 if QKR else bf16,
                                 tag="wq")
            wk_sb = singles.tile([128, 3, NF, 256], fp8 if QKR else bf16,
                                 tag="wk")
            for dst, src in ((wq_sb, wq), (wk_sb, wk)):
                nc.sync.dma_start(
                    out=dst,
                    in_=src.rearrange("v (nf p) d -> p v nf d", p=128),
                )
            cos_sb = singles.tile([128, T], bf16, tag="cos")
            sin_sb = singles.tile([128, T], bf16, tag="sin")
            nc.scalar.dma_start(out=cos_sb, in_=cosd[:, :])
            nc.scalar.dma_start(out=sin_sb, in_=sind[:, :])
            wv_sb = singles.tile([128, 3, NF, 256], fp8 if VR else bf16,
                                 tag="wv")
            nc.scalar.dma_start(
                out=wv_sb,
                in_=wv.rearrange("v (nf p) d -> p v nf d", p=128),
            )
            mask_sb = singles.tile([128, 4, 512], bf16, tag="mask")
            nc.scalar.dma_start(
                out=mask_sb, in_=maskd.rearrange("p (v t) -> p v t", v=4)
            )
            wo_sb = singles.tile([128, 2, HID], bf16, tag="wo")
            nc.scalar.dma_start(
                out=wo_sb, in_=wo.rearrange("(kt p) f -> p kt f", p=128)
            )
            ones_a = singles.tile([128, 2], bf16, tag="onesa")  # [1, 0]
            ones_b = singles.tile([128, 2], bf16, tag="onesb")  # [1, 1]
            nc.vector.memset(ones_a[:, 0:1], 1.0)
            nc.vector.memset(ones_a[:, 1:2], 0.0)
            nc.vector.memset(ones_b, 1.0)

            # --- residents ---
            qt = [res.tile([128, BT], bf16, tag=f"qt{h}", name=f"qt{h}") for h in range(HL)]
            kt = [res.tile([128, BT], bf16, tag=f"kt{h}", name=f"kt{h}") for h in range(HL)]
            vres = res.tile([128, 32, 256], bf16, tag="vres")
            ot = [res.tile([128, T], bf16, tag=f"ot{p}", name=f"ot{p}") for p in range(4)]

            _ph = os.environ.get("BK_PHASES", "123")
            # ================= Phase 1: QKV projections + RoPE =================
            with (
                tc.tile_pool(name="hsp", bufs=B_HSP) as hsp,
                tc.tile_pool(name="rope", bufs=B_ROPE) as rope,
                tc.tile_pool(name="qkps", bufs=B_QK, space="PSUM") as qkps,
                tc.tile_pool(name="vps", bufs=B_VPS, space="PSUM") as vps,
            ):
                for c in (range(8) if "1" in _ph else []):
                    cs = slice(c * 512, (c + 1) * 512)
                    hs_t = hsp.tile([128, 3, NF, 512], fp8, tag="hs")
                    if any_r:
                        nc.sync.dma_start(
                            out=hs_t,
                            in_=hs3[:, :, cs].rearrange(
                                "v (nf p) t -> p v nf t", p=128),
                        )
                    if not (QKR and VR):
                        hs16_t = hsp.tile([128, NF, 512], bf16, tag="hs16")
                        nc.sync.dma_start(
                            out=hs16_t,
                            in_=hsT16[:, cs].rearrange(
                                "(nf p) t -> p nf t", p=128),
                        )
                    for h in range(HL):
                        for w_sb, dest in ((wq_sb, qt[h]), (wk_sb, kt[h])):
                            ps = qkps.tile([128, 512], f32, tag="qk")
                            if QKR:
                                n = 0
                                for v in range(3):
                                    for fp_ in range(NF // 2):
                                        fsl = slice(2 * fp_, 2 * fp_ + 2)
                                        nc.tensor.matmul(
                                            ps,
                                            lhsT=w_sb[:, v, fsl,
                                                      h * 128:(h + 1) * 128],
                                            rhs=hs_t[:, v, fsl, :],
                                            start=(n == 0),
                                            stop=(n == 3 * (NF // 2) - 1),
                                            perf_mode=DR,
                                        )
                                        n += 1
                            else:
                                for f in range(NF):
                                    nc.tensor.matmul(
                                        ps,
                                        lhsT=w_sb[:, 0, f, h * 128:(h + 1) * 128],
                                        rhs=hs16_t[:, f, :],
                                        start=(f == 0),
                                        stop=(f == NF - 1),
                                    )
                            tl = slice((c % 4) * 512, (c % 4) * 512 + 512)
                            qf = rope.tile([128, 512], bf16, tag="qf")
                            nc.scalar.copy(qf, ps)
                            rot = rope.tile([128, 512], bf16, tag="rot")
                            nc.gpsimd.dma_start(out=rot[0:64, :], in_=qf[64:128, :])
                            nc.gpsimd.dma_start(out=rot[64:128, :], in_=qf[0:64, :])
                            t1 = rope.tile([128, 512], bf16, tag="t1")
                            nc.vector.tensor_mul(t1, rot, sin_sb[:, tl])
                            t2 = rope.tile([128, 512], bf16, tag="t2")
                            nc.vector.tensor_mul(t2, qf, cos_sb[:, tl])
                            nc.vector.tensor_add(dest[:, cs], t1, t2)
                    for s in range(4):
                        vp = vps.tile([128, 256], f32, tag="v")
                        if VR:
                            n = 0
                            for v in range(3):
                                for fp_ in range(NF // 2):
                                    fsl = slice(2 * fp_, 2 * fp_ + 2)
                                    nc.tensor.matmul(
                                        vp,
                                        lhsT=hs_t[:, v, fsl, s * 128:(s + 1) * 128],
                                        rhs=wv_sb[:, v, fsl, :],
                                        start=(n == 0),
                                        stop=(n == 3 * (NF // 2) - 1),
                                        perf_mode=DR,
                                    )
                                    n += 1
                        else:
                            for f in range(NF):
                                nc.tensor.matmul(
                                    vp,
                                    lhsT=hs16_t[:, f, s * 128:(s + 1) * 128],
                                    rhs=wv_sb[:, 0, f, :],
                                    start=(f == 0),
                                    stop=(f == NF - 1),
                                )
                        nc.scalar.activation(
                            vres[:, c * 4 + s, :], vp, COPY,
                            scale=(1.0 / WSCALE) if VR else 1.0,
                        )

            # ========== Phase 2+3: attention + interleaved o_proj ==========
            with (
                tc.tile_pool(name="sps", bufs=B_SPS, space="PSUM") as sps,
                tc.tile_pool(name="ops", bufs=B_OPS, space="PSUM") as ops,
                tc.tile_pool(name="smps", bufs=B_SM, space="PSUM") as smps,
                tc.tile_pool(name="pt", bufs=4) as ptp,
                tc.tile_pool(name="att_sm", bufs=4) as atsm,
                tc.tile_pool(name="ob", bufs=4) as obp,
            ):
                def attn_chunk(p, c):
                    b, h = p // 2, p % 2
                    jmax = 4 * (c + 1)
                    mmax = jmax // 2
                    cl = slice(c * 512, (c + 1) * 512)
                    qtb, ktb = qt[h], kt[h]
                    o_ps = ops.tile([128, 512], f32, tag="o")
                    sm_ps = smps.tile([2, 512], f32, tag="sm")

                    def emit_s2(m):
                        sp = sps.tile([128, 2, 512], f32, tag="s")
                        for i, j in ((0, 2 * m), (1, 2 * m + 1)):
                            nc.tensor.matmul(
                                sp[:, i, :],
                                lhsT=ktb[:, b * T + j * 128: b * T + (j + 1) * 128],
                                rhs=qtb[:, b * T + c * 512: b * T + (c + 1) * 512],
                                start=True, stop=True,
                            )
                        return sp

                    def emit_epv2(m, sp):
                        pt_t = ptp.tile([128, 2, 512], bf16, tag="p")
                        nc.scalar.activation(pt_t, sp, EXP, scale=SCALE)
                        for i, j in ((0, 2 * m), (1, 2 * m + 1)):
                            pth = pt_t[:, i, :]
                            if j >= 4 * c:
                                nc.vector.tensor_mul(pth, pth, mask_sb[:, j - 4 * c, :])
                            nc.tensor.matmul(
                                o_ps,
                                lhsT=vres[:, b * 16 + j, h * 128:(h + 1) * 128],
                                rhs=pth,
                                start=(j == 0), stop=(j == jmax - 1),
                            )
                            nc.tensor.matmul(
                                sm_ps,
                                lhsT=(ones_b if j >= 12 else ones_a),
                                rhs=pth,
                                start=(j == 0), stop=(j == jmax - 1),
                            )

                    sq = [emit_s2(0)]
                    for m in range(mmax):
                        if m + 1 < mmax:
                            sq.append(emit_s2(m + 1))
                        emit_epv2(m, sq[m])

                    rec = atsm.tile([1, 512], f32, tag="rec")
                    nc.vector.reciprocal(rec, sm_ps[0:1, :])
                    bc = atsm.tile([128, 512], f32, tag="bc")
                    nc.gpsimd.partition_broadcast(bc, rec)
                    nc.vector.tensor_mul(ot[p][:, cl], o_ps, bc)
                    sm_sb = atsm.tile([2, 512], f32, tag="smsb")
                    nc.vector.tensor_copy(sm_sb, sm_ps)
                    nc.sync.dma_start(out=sumsd[p, :, cl], in_=sm_sb)

                def oproj_chunk(b, c):
                    tl = slice(c * 512, (c + 1) * 512)
                    cg = slice((b * 4 + c) * 512, (b * 4 + c + 1) * 512)
                    for g in range(4):
                        ob = obp.tile([128, 4, 512], bf16, tag="ob")
                        for fi in range(4):
                            fo = g * 4 + fi
                            fs = slice(fo * 128, (fo + 1) * 128)
                            pp = ops.tile([128, 512], f32, tag="o")
                            nc.tensor.matmul(
                                pp, lhsT=wo_sb[:, 0, fs], rhs=ot[b * 2 + 0][:, tl],
                                start=True, stop=False,
                            )
                            nc.tensor.matmul(
                                pp, lhsT=wo_sb[:, 1, fs], rhs=ot[b * 2 + 1][:, tl],
                                start=False, stop=True,
                            )
                            if fo % 2 == 0:
                                nc.vector.tensor_copy(ob[:, fi, :], pp)
                            else:
                                nc.scalar.copy(ob[:, fi, :], pp)
                        fg = slice(g * 4, g * 4 + 4)
                        nc.gpsimd.dma_start(
                            out=outT[:, cg].rearrange(
                                "(nf p) t -> p nf t", p=128)[:, fg, :],
                            in_=ob,
                        )

                if "2" in _ph:
                    pending = []
                    for b in range(B):
                        for c in range(4):
                            attn_chunk(b * 2 + 0, c)
                            attn_chunk(b * 2 + 1, c)
                            if "3" in _ph:
                                if LAG:
                                    if pending:
                                        oproj_chunk(*pending.pop())
                                    pending.append((b, c))
                                else:
                                    oproj_chunk(b, c)
                    if "3" in _ph and LAG:
                        while pending:
                            oproj_chunk(*pending.pop())

    nc.compile()
    return nc


def _get_nc():
    if "nc" not in _NC_CACHE:
        _NC_CACHE["nc"] = build_nc()
    return _NC_CACHE["nc"]


def _fp8_triple(w):
    """w [rows, cols] fp32 -> stacked [3, rows, cols] fp8:
    (fp8(64w), fp8(8w), fp8(8*(64w - fp8(64w))))."""
    w64 = (w * WSCALE).astype(E4)
    w8 = (w * 8.0).astype(E4)
    wres8 = (8.0 * (w * WSCALE - w64.astype(np.float32))).astype(E4)
    return np.stack([w64, w8, wres8])


def _host_inputs(hidden_states, q_w, k_w, v_w, o_w):
    """Per-core input dicts."""
    hs2d = np.ascontiguousarray(hidden_states.reshape(BT, HID).T)
    inv = 10000.0 ** (-np.arange(64, dtype=np.float64) / 64.0)
    t = np.arange(T, dtype=np.float64)
    fr = t[None, :] * inv[:, None]  # [64, T]
    tbl_scale = (1.0 / WSCALE) if QKR else 1.0
    cosT = (np.cos(np.concatenate([fr, fr], 0)) * tbl_scale).astype(BF16)
    sinT = (np.sin(np.concatenate([fr, fr], 0)) * tbl_scale).astype(np.float64)
    sinT[:64] *= -1.0  # sign-baked for swap-halves rotate
    sinT = sinT.astype(BF16)
    masks = np.zeros((128, 4, 512), dtype=np.float32)
    kk = np.arange(128)[:, None]
    tt = np.arange(512)[None, :]
    for v in range(4):
        masks[:, v, :] = (tt >= 128 * v + kk).astype(np.float32)
    masks = masks.reshape(128, 2048).astype(BF16)

    # hs triple: hi = fp8(hs), lo8 = fp8(8*(hs - hi)), d8 = fp8(hs/8)
    hs_hi = hs2d.astype(E4)
    hs_lo8 = (8.0 * (hs2d - hs_hi.astype(np.float32))).astype(E4)
    hs_d8 = (hs2d / 8.0).astype(E4)
    hs3 = np.stack([hs_hi, hs_lo8, hs_d8])

    base = {
        "cosT": cosT,
        "sinT": sinT,
        "masks": masks,
        "hs3": hs3,
    }
    if not (QKR and VR):
        base["hsT16"] = hs2d.astype(BF16)

    in_maps = []
    for core in range(NCORES):
        rs = slice(core * 256, (core + 1) * 256)
        m = dict(base)
        qs = np.ascontiguousarray(q_w[rs, :].T)
        ks = np.ascontiguousarray(k_w[rs, :].T)
        vs = np.ascontiguousarray(v_w[rs, :].T)
        if QKR:
            m["wq"] = _fp8_triple(qs)
            m["wk"] = _fp8_triple(ks)
        else:
            m["wq"] = np.stack([qs.astype(BF16)] * 3)
            m["wk"] = np.stack([ks.astype(BF16)] * 3)
        if VR:
            m["wv"] = _fp8_triple(vs)
        else:
            m["wv"] = np.stack([vs.astype(BF16)] * 3)
        m["wo"] = np.ascontiguousarray(o_w[:, rs].T).astype(BF16)
        in_maps.append(m)
    return in_maps


def _host_m(hidden_states, q_w, k_w, rowsum_last):
    """Exact CaM bernoulli decisions. Only needs last-row scores for the
    513 tail keys; the softmax denominator cancels in the prob ratio (the
    device rowsum is used only for the far-from-binding 1e-6 floor)."""
    hs = hidden_states.astype(np.float64)
    inv = 10000.0 ** (-np.arange(64, dtype=np.float64) / 64.0)

    def rope_rows(X, pos):  # X [N, H, 128], pos [N]
        fr = pos[:, None] * inv[None, :]  # [N, 64]
        cos = np.cos(np.concatenate([fr, fr], -1))[:, None, :]
        sin = np.sin(np.concatenate([fr, fr], -1))[:, None, :]
        rot = np.concatenate([-X[..., 64:], X[..., :64]], -1)
        return X * cos + rot * sin

    qW = q_w.astype(np.float64)
    kW = k_w.astype(np.float64)
    m = np.zeros((B, H), np.float64)
    for b in range(B):
        q_last = (hs[b, T - 1] @ qW.T).reshape(1, H, D)
        ql = rope_rows(q_last, np.array([float(T - 1)]))[0]  # [H, D]
        Kt = (hs[b, EVICT:] @ kW.T).reshape(T - EVICT, H, D)
        Kt = rope_rows(Kt, np.arange(EVICT, T, dtype=np.float64))
        s = np.einsum('khd,hd->hk', Kt, ql) * SCALE  # [H, 513]
        a = np.exp(s)
        for h in range(H):
            rs = max(float(rowsum_last[b, h]), 1e-30)
            avg_w = max(float(np.mean(a[h, 1:])) / rs, 1e-6)
            prob = float(np.clip((a[h, 0] / rs) / avg_w, 0.0, 1.0))
            m[b, h] = 1.0 if U_CONST[b, h] < prob else 0.0
    return m


def _epilogue(out, results, hidden_states, q_w, k_w, v_w, o_w):
    """Add the CaM rank-1 correction per (b, h) on host."""
    rowsum_last = np.zeros((B, H), np.float64)
    for core in range(NCORES):
        r = results[core]
        for p in range(4):
            b, hl = p // 2, p % 2
            h = core * HL + hl
            rowsum_last[b, h] = float(r["sums"][p][0][T - 1])
    m_tbl = _host_m(hidden_states, q_w, k_w, rowsum_last)
    for core in range(NCORES):
        r = results[core]
        for p in range(4):
            b, hl = p // 2, p % 2
            h = core * HL + hl
            if m_tbl[b, h] == 0.0:
                continue
            rowsum = np.asarray(r["sums"][p][0], np.float64)
            tails = np.asarray(r["sums"][p][1], np.float64)
            # exact v_e from fp32 inputs
            v_row = hidden_states[b, EVICT, :] @ v_w[h * D:(h + 1) * D, :].T
            v_e = v_row * (1.0 / RB)  # [D]
            w_e = o_w[:, h * D:(h + 1) * D] @ v_e  # [HID]
            s_tail = (tails / np.maximum(rowsum, 1e-30)).astype(np.float32)
            out[b] += np.outer(s_tail, w_e).astype(np.float32)
    return out


def kernel(hidden_states, attention_mask, q_w, k_w, v_w, o_w):
    from concourse.bass_utils import run_bass_kernel_spmd

    nc = _get_nc()
    in_maps = _host_inputs(hidden_states, q_w, k_w, v_w, o_w)
    trace = bool(int(os.environ.get("BK_TRACE", "0")))
    res = run_bass_kernel_spmd(
        nc, in_maps, core_ids=list(range(NCORES)), trace=trace,
    )
    if trace and res.exec_time_ns is not None:
        print(f"HW exec time: {res.exec_time_ns} ns")
        _NC_CACHE["last_exec_ns"] = res.exec_time_ns
        _NC_CACHE["last_trace"] = res.instructions_and_trace
    results = res.results

    acc = np.zeros((HID, BT), dtype=np.float32)
    for core in range(NCORES):
        acc += np.asarray(results[core]["outT"], np.float32)
    out = np.ascontiguousarray(acc.T).reshape(B, T, HID)
    out = _epilogue(out, results, hidden_states, q_w, k_w, v_w, o_w)
    return out.astype(np.float32)


# revision 20
# speedup vs baseline: 1.1510x; 1.0165x over previous
"""Trainium2 Bass kernel for nn_LlamaAttention_cam (sparse_attention).

Sharding: 16 heads across 8 cores (2 heads/core), both batches per core.
Q/K/V projections column-parallel over heads; o_proj row-parallel (per-core
partial outputs summed on host). The CaM merge is a rank-1 correction
(s_tail outer v_e) applied on host from tiny device-side statistics; the
bernoulli decision is recomputed exactly on host (needs only last-row
scores for the 513 tail keys — the softmax denominator cancels).

Projections run as 3-term residual fp8 DoubleRow matmuls:
  64*W*hs = W64(x)hs_hi + W8(x)hs_lo8 + Wres8(x)hs_d8
with W64=fp8(64W), W8=fp8(8W), Wres8=fp8(8(64W-W64)), hs_hi=fp8(hs),
hs_lo8=fp8(8(hs-hs_hi)), hs_d8=fp8(hs/8). Each term pairs two 128-deep
k-subtiles per DoubleRow instruction, so the projection costs 0.75x of
bf16 while retaining ~bf16 accuracy. Scores / P@V / row-sum / o_proj stay
bf16 (fp8 there fails the accuracy gate: softmax output is a near-zero-
mean average, so per-element fp8 noise does not average down).

Self-contained: hardcodes all shapes; takes full inputs, returns full output.
"""

import math
import os

import numpy as np
import ml_dtypes

B, T, HID, H = 2, 2048, 2048, 16
D = 128
NCORES = 8
HL = H // NCORES  # heads per core = 2
BT = B * T  # 4096
NF = HID // 128  # 16 f-tiles
SCALE = 1.0 / math.sqrt(D)
RB = int(0.25 * T)  # 512 recent budget
WS = T - RB  # 1536
EVICT = WS - 1  # 1535
WSCALE = 64.0  # fp8 weight pre-scale

# jax.random.uniform(jax.random.key(42), (2,16), float32); bernoulli(key,p) == u < p
U_CONST = np.array(
    [[0.59400654, 0.43801308, 0.6285691, 0.00791204, 0.27834702,
      0.7976179, 0.8521497, 0.9625306, 0.67656493, 0.11104441,
      0.4959929, 0.7311437, 0.18970704, 0.1544199, 0.03802836,
      0.33559263],
     [0.92825687, 0.6123972, 0.49262476, 0.733806, 0.18920851,
      0.15386605, 0.037136197, 0.32930005, 0.9372028, 0.5957513,
      0.4615929, 0.6695677, 0.07019377, 0.39408123, 0.55786455,
      0.35412872]], dtype=np.float32)

BF16 = ml_dtypes.bfloat16
E4 = ml_dtypes.float8_e4m3

_NC_CACHE = {}


def _flag(name, default):
    return os.environ.get(name, default) == "1"


QKR = _flag("BK_QKR", "1")   # Q/K projections residual-fp8 DoubleRow
VR = _flag("BK_VR", "1")     # V projection residual-fp8 DoubleRow
LAG = _flag("BK_LAG", "1")   # o_proj emission lags attn by one chunk


def build_nc():
    import concourse.bacc as bacc
    import concourse.mybir as mybir
    import concourse.tile as tile

    f32 = mybir.dt.float32
    bf16 = mybir.dt.bfloat16
    fp8 = mybir.dt.float8e4
    EXP = mybir.ActivationFunctionType.Exp
    COPY = mybir.ActivationFunctionType.Copy
    DR = mybir.MatmulPerfMode.DoubleRow

    nc = bacc.Bacc("TRN2", target_bir_lowering=False, debug=False)
    env = os.environ
    B_QK = int(env.get("BK_QKPS", "4")); B_VPS = int(env.get("BK_VPS", "2"))
    B_HSP = int(env.get("BK_HSP", "2")); B_SPS = int(env.get("BK_SPS", "2"))
    B_OPS = int(env.get("BK_OPS", "3")); B_SM = int(env.get("BK_SM", "1"))
    B_ROPE = int(env.get("BK_ROPE", "3"))

    any_r = QKR or VR
    hs3 = nc.dram_tensor("hs3", [3, HID, BT], fp8, kind="ExternalInput")
    if not (QKR and VR):
        hsT16 = nc.dram_tensor("hsT16", [HID, BT], bf16, kind="ExternalInput")
    wq = nc.dram_tensor("wq", [3, HID, 256], fp8 if QKR else bf16,
                        kind="ExternalInput")
    wk = nc.dram_tensor("wk", [3, HID, 256], fp8 if QKR else bf16,
                        kind="ExternalInput")
    wv = nc.dram_tensor("wv", [3, HID, 256], fp8 if VR else bf16,
                        kind="ExternalInput")
    wo = nc.dram_tensor("wo", [256, HID], bf16, kind="ExternalInput")
    cosd = nc.dram_tensor("cosT", [128, T], bf16, kind="ExternalInput")
    sind = nc.dram_tensor("sinT", [128, T], bf16, kind="ExternalInput")
    maskd = nc.dram_tensor("masks", [128, 2048], bf16, kind="ExternalInput")

    outT = nc.dram_tensor("outT", [HID, BT], bf16, kind="ExternalOutput")
    sumsd = nc.dram_tensor("sums", [4, 2, T], f32, kind="ExternalOutput")

    with tile.TileContext(nc) as tc:
        with (
            tc.tile_pool(name="singles", bufs=1) as singles,
            tc.tile_pool(name="res", bufs=1) as res,
        ):
            # --- constant loads: wq/wk on SP queue; the rest on Act queue ---
            # weights + tables on the Act HWDGE queue (Act engine is idle at
            # startup); hs chunks stream on the SP queue in parallel
            wq_sb = singles.tile([128, 3, NF, 256], fp8 if QKR else bf16,
                                 tag="wq")
            wk_sb = singles.tile([128, 3, NF, 256], fp8 if QKR else bf16,
                                 tag="wk")
            for dst, src in ((wq_sb, wq), (wk_sb, wk)):
                nc.scalar.dma_start(
                    out=dst,
                    in_=src.rearrange("v (nf p) d -> p v nf d", p=128),
                )
            wv_sb = singles.tile([128, 3, NF, 256], fp8 if VR else bf16,
                                 tag="wv")
            nc.scalar.dma_start(
                out=wv_sb,
                in_=wv.rearrange("v (nf p) d -> p v nf d", p=128),
            )
            cos_sb = singles.tile([128, T], bf16, tag="cos")
            sin_sb = singles.tile([128, T], bf16, tag="sin")
            nc.scalar.dma_start(out=cos_sb, in_=cosd[:, :])
            nc.scalar.dma_start(out=sin_sb, in_=sind[:, :])
            mask_sb = singles.tile([128, 4, 512], bf16, tag="mask")
            wo_sb = singles.tile([128, 2, HID], bf16, tag="wo")
            ones_a = singles.tile([128, 2], bf16, tag="onesa")  # [1, 0]
            ones_b = singles.tile([128, 2], bf16, tag="onesb")  # [1, 1]
            nc.vector.memset(ones_a[:, 0:1], 1.0)
            nc.vector.memset(ones_a[:, 1:2], 0.0)
            nc.vector.memset(ones_b, 1.0)

            # --- residents ---
            qt = [res.tile([128, BT], bf16, tag=f"qt{h}", name=f"qt{h}") for h in range(HL)]
            kt = [res.tile([128, BT], bf16, tag=f"kt{h}", name=f"kt{h}") for h in range(HL)]
            vres = res.tile([128, 32, 256], bf16, tag="vres")
            ot = [res.tile([128, T], bf16, tag=f"ot{p}", name=f"ot{p}") for p in range(4)]

            _ph = os.environ.get("BK_PHASES", "123")
            # ================= Phase 1: QKV projections + RoPE =================
            with (
                tc.tile_pool(name="hsp", bufs=B_HSP) as hsp,
                tc.tile_pool(name="rope", bufs=B_ROPE) as rope,
                tc.tile_pool(name="qkps", bufs=B_QK, space="PSUM") as qkps,
                tc.tile_pool(name="vps", bufs=B_VPS, space="PSUM") as vps,
            ):
                for c in (range(8) if "1" in _ph else []):
                    cs = slice(c * 512, (c + 1) * 512)
                    hs_t = hsp.tile([128, 3, NF, 512], fp8, tag="hs")
                    if any_r:
                        if c == 0:
                            # split by fp8-version so the first matmul group
                            # starts as soon as version 0 lands
                            for v in range(3):
                                nc.sync.dma_start(
                                    out=hs_t[:, v:v + 1],
                                    in_=hs3[v:v + 1, :, cs].rearrange(
                                        "v (nf p) t -> p v nf t", p=128),
                                )
                        else:
                            nc.sync.dma_start(
                                out=hs_t,
                                in_=hs3[:, :, cs].rearrange(
                                    "v (nf p) t -> p v nf t", p=128),
                            )
                    if not (QKR and VR):
                        hs16_t = hsp.tile([128, NF, 512], bf16, tag="hs16")
                        nc.sync.dma_start(
                            out=hs16_t,
                            in_=hsT16[:, cs].rearrange(
                                "(nf p) t -> p nf t", p=128),
                        )
                    for h in range(HL):
                        for w_sb, dest in ((wq_sb, qt[h]), (wk_sb, kt[h])):
                            ps = qkps.tile([128, 512], f32, tag="qk")
                            if QKR:
                                n = 0
                                for v in range(3):
                                    for fp_ in range(NF // 2):
                                        fsl = slice(2 * fp_, 2 * fp_ + 2)
                                        nc.tensor.matmul(
                                            ps,
                                            lhsT=w_sb[:, v, fsl,
                                                      h * 128:(h + 1) * 128],
                                            rhs=hs_t[:, v, fsl, :],
                                            start=(n == 0),
                                            stop=(n == 3 * (NF // 2) - 1),
                                            perf_mode=DR,
                                        )
                                        n += 1
                            else:
                                for f in range(NF):
                                    nc.tensor.matmul(
                                        ps,
                                        lhsT=w_sb[:, 0, f, h * 128:(h + 1) * 128],
                                        rhs=hs16_t[:, f, :],
                                        start=(f == 0),
                                        stop=(f == NF - 1),
                                    )
                            tl = slice((c % 4) * 512, (c % 4) * 512 + 512)
                            qf = rope.tile([128, 512], bf16, tag="qf")
                            nc.scalar.copy(qf, ps)
                            rot = rope.tile([128, 512], bf16, tag="rot")
                            nc.gpsimd.dma_start(out=rot[0:64, :], in_=qf[64:128, :])
                            nc.gpsimd.dma_start(out=rot[64:128, :], in_=qf[0:64, :])
                            t1 = rope.tile([128, 512], bf16, tag="t1")
                            nc.vector.tensor_mul(t1, rot, sin_sb[:, tl])
                            t2 = rope.tile([128, 512], bf16, tag="t2")
                            nc.vector.tensor_mul(t2, qf, cos_sb[:, tl])
                            nc.vector.tensor_add(dest[:, cs], t1, t2)
                    for s in range(4):
                        vp = vps.tile([128, 256], f32, tag="v")
                        if VR:
                            n = 0
                            for v in range(3):
                                for fp_ in range(NF // 2):
                                    fsl = slice(2 * fp_, 2 * fp_ + 2)
                                    nc.tensor.matmul(
                                        vp,
                                        lhsT=hs_t[:, v, fsl, s * 128:(s + 1) * 128],
                                        rhs=wv_sb[:, v, fsl, :],
                                        start=(n == 0),
                                        stop=(n == 3 * (NF // 2) - 1),
                                        perf_mode=DR,
                                    )
                                    n += 1
                        else:
                            for f in range(NF):
                                nc.tensor.matmul(
                                    vp,
                                    lhsT=hs16_t[:, f, s * 128:(s + 1) * 128],
                                    rhs=wv_sb[:, 0, f, :],
                                    start=(f == 0),
                                    stop=(f == NF - 1),
                                )
                        nc.scalar.activation(
                            vres[:, c * 4 + s, :], vp, COPY,
                            scale=(1.0 / WSCALE) if VR else 1.0,
                        )
                # phase-2/3 constants: SP queue, after the hs stream
                nc.sync.dma_start(
                    out=mask_sb, in_=maskd.rearrange("p (v t) -> p v t", v=4)
                )
                nc.sync.dma_start(
                    out=wo_sb, in_=wo.rearrange("(kt p) f -> p kt f", p=128)
                )

            # ========== Phase 2+3: attention + interleaved o_proj ==========
            with (
                tc.tile_pool(name="sps", bufs=B_SPS, space="PSUM") as sps,
                tc.tile_pool(name="ops", bufs=B_OPS, space="PSUM") as ops,
                tc.tile_pool(name="smps", bufs=B_SM, space="PSUM") as smps,
                tc.tile_pool(name="pt", bufs=4) as ptp,
                tc.tile_pool(name="att_sm", bufs=4) as atsm,
                tc.tile_pool(name="ob", bufs=4) as obp,
            ):
                def attn_chunk(p, c):
                    b, h = p // 2, p % 2
                    jmax = 4 * (c + 1)
                    mmax = jmax // 2
                    cl = slice(c * 512, (c + 1) * 512)
                    qtb, ktb = qt[h], kt[h]
                    o_ps = ops.tile([128, 512], f32, tag="o")
                    sm_ps = smps.tile([2, 512], f32, tag="sm")

                    def emit_s2(m):
                        sp = sps.tile([128, 2, 512], f32, tag="s")
                        for i, j in ((0, 2 * m), (1, 2 * m + 1)):
                            nc.tensor.matmul(
                                sp[:, i, :],
                                lhsT=ktb[:, b * T + j * 128: b * T + (j + 1) * 128],
                                rhs=qtb[:, b * T + c * 512: b * T + (c + 1) * 512],
                                start=True, stop=True,
                            )
                        return sp

                    def emit_epv2(m, sp):
                        pt_t = ptp.tile([128, 2, 512], bf16, tag="p")
                        nc.scalar.activation(pt_t, sp, EXP, scale=SCALE)
                        for i, j in ((0, 2 * m), (1, 2 * m + 1)):
                            pth = pt_t[:, i, :]
                            if j >= 4 * c:
                                nc.vector.tensor_mul(pth, pth, mask_sb[:, j - 4 * c, :])
                            nc.tensor.matmul(
                                o_ps,
                                lhsT=vres[:, b * 16 + j, h * 128:(h + 1) * 128],
                                rhs=pth,
                                start=(j == 0), stop=(j == jmax - 1),
                            )
                            nc.tensor.matmul(
                                sm_ps,
                                lhsT=(ones_b if j >= 12 else ones_a),
                                rhs=pth,
                                start=(j == 0), stop=(j == jmax - 1),
                            )

                    sq = [emit_s2(0)]
                    for m in range(mmax):
                        if m + 1 < mmax:
                            sq.append(emit_s2(m + 1))
                        emit_epv2(m, sq[m])

                    rec = atsm.tile([1, 512], f32, tag="rec")
                    nc.vector.reciprocal(rec, sm_ps[0:1, :])
                    bc = atsm.tile([128, 512], f32, tag="bc")
                    nc.gpsimd.partition_broadcast(bc, rec)
                    nc.vector.tensor_mul(ot[p][:, cl], o_ps, bc)
                    sm_sb = atsm.tile([2, 512], f32, tag="smsb")
                    nc.vector.tensor_copy(sm_sb, sm_ps)
                    nc.gpsimd.dma_start(out=sumsd[p, :, cl], in_=sm_sb)

                def oproj_chunk(b, c):
                    tl = slice(c * 512, (c + 1) * 512)
                    cg = slice((b * 4 + c) * 512, (b * 4 + c + 1) * 512)
                    for g in range(4):
                        ob = obp.tile([128, 4, 512], bf16, tag="ob")
                        for fi in range(4):
                            fo = g * 4 + fi
                            fs = slice(fo * 128, (fo + 1) * 128)
                            pp = ops.tile([128, 512], f32, tag="o")
                            nc.tensor.matmul(
                                pp, lhsT=wo_sb[:, 0, fs], rhs=ot[b * 2 + 0][:, tl],
                                start=True, stop=False,
                            )
                            nc.tensor.matmul(
                                pp, lhsT=wo_sb[:, 1, fs], rhs=ot[b * 2 + 1][:, tl],
                                start=False, stop=True,
                            )
                            if fo % 2 == 0:
                                nc.vector.tensor_copy(ob[:, fi, :], pp)
                            else:
                                nc.scalar.copy(ob[:, fi, :], pp)
                        fg = slice(g * 4, g * 4 + 4)
                        nc.sync.dma_start(
                            out=outT[:, cg].rearrange(
                                "(nf p) t -> p nf t", p=128)[:, fg, :],
                            in_=ob,
                        )

                if "2" in _ph:
                    pending = []
                    for b in range(B):
                        for c in range(4):
                            attn_chunk(b * 2 + 0, c)
                            attn_chunk(b * 2 + 1, c)
                            if "3" in _ph:
                                if LAG:
                                    if pending:
                                        oproj_chunk(*pending.pop())
                                    pending.append((b, c))
                                else:
                                    oproj_chunk(b, c)
                    if "3" in _ph and LAG:
                        while pending:
                            oproj_chunk(*pending.pop())

    nc.compile()
    return nc


def _get_nc():
    if "nc" not in _NC_CACHE:
        _NC_CACHE["nc"] = build_nc()
    return _NC_CACHE["nc"]


def _fp8_triple(w):
    """w [rows, cols] fp32 -> stacked [3, rows, cols] fp8:
    (fp8(64w), fp8(8w), fp8(8*(64w - fp8(64w))))."""
    w64 = (w * WSCALE).astype(E4)
    w8 = (w * 8.0).astype(E4)
    wres8 = (8.0 * (w * WSCALE - w64.astype(np.float32))).astype(E4)
    return np.stack([w64, w8, wres8])


def _host_inputs(hidden_states, q_w, k_w, v_w, o_w):
    """Per-core input dicts."""
    hs2d = np.ascontiguousarray(hidden_states.reshape(BT, HID).T)
    inv = 10000.0 ** (-np.arange(64, dtype=np.float64) / 64.0)
    t = np.arange(T, dtype=np.float64)
    fr = t[None, :] * inv[:, None]  # [64, T]
    tbl_scale = (1.0 / WSCALE) if QKR else 1.0
    cosT = (np.cos(np.concatenate([fr, fr], 0)) * tbl_scale).astype(BF16)
    sinT = (np.sin(np.concatenate([fr, fr], 0)) * tbl_scale).astype(np.float64)
    sinT[:64] *= -1.0  # sign-baked for swap-halves rotate
    sinT = sinT.astype(BF16)
    masks = np.zeros((128, 4, 512), dtype=np.float32)
    kk = np.arange(128)[:, None]
    tt = np.arange(512)[None, :]
    for v in range(4):
        masks[:, v, :] = (tt >= 128 * v + kk).astype(np.float32)
    masks = masks.reshape(128, 2048).astype(BF16)

    # hs triple: hi = fp8(hs), lo8 = fp8(8*(hs - hi)), d8 = fp8(hs/8)
    hs_hi = hs2d.astype(E4)
    hs_lo8 = (8.0 * (hs2d - hs_hi.astype(np.float32))).astype(E4)
    hs_d8 = (hs2d / 8.0).astype(E4)
    hs3 = np.stack([hs_hi, hs_lo8, hs_d8])

    base = {
        "cosT": cosT,
        "sinT": sinT,
        "masks": masks,
        "hs3": hs3,
    }
    if not (QKR and VR):
        base["hsT16"] = hs2d.astype(BF16)

    in_maps = []
    for core in range(NCORES):
        rs = slice(core * 256, (core + 1) * 256)
        m = dict(base)
        qs = np.ascontiguousarray(q_w[rs, :].T)
        ks = np.ascontiguousarray(k_w[rs, :].T)
        vs = np.ascontiguousarray(v_w[rs, :].T)
        if QKR:
            m["wq"] = _fp8_triple(qs)
            m["wk"] = _fp8_triple(ks)
        else:
            m["wq"] = np.stack([qs.astype(BF16)] * 3)
            m["wk"] = np.stack([ks.astype(BF16)] * 3)
        if VR:
            m["wv"] = _fp8_triple(vs)
        else:
            m["wv"] = np.stack([vs.astype(BF16)] * 3)
        m["wo"] = np.ascontiguousarray(o_w[:, rs].T).astype(BF16)
        in_maps.append(m)
    return in_maps


def _host_m(hidden_states, q_w, k_w, rowsum_last):
    """Exact CaM bernoulli decisions. Only needs last-row scores for the
    513 tail keys; the softmax denominator cancels in the prob ratio (the
    device rowsum is used only for the far-from-binding 1e-6 floor)."""
    hs = hidden_states.astype(np.float64)
    inv = 10000.0 ** (-np.arange(64, dtype=np.float64) / 64.0)

    def rope_rows(X, pos):  # X [N, H, 128], pos [N]
        fr = pos[:, None] * inv[None, :]  # [N, 64]
        cos = np.cos(np.concatenate([fr, fr], -1))[:, None, :]
        sin = np.sin(np.concatenate([fr, fr], -1))[:, None, :]
        rot = np.concatenate([-X[..., 64:], X[..., :64]], -1)
        return X * cos + rot * sin

    qW = q_w.astype(np.float64)
    kW = k_w.astype(np.float64)
    m = np.zeros((B, H), np.float64)
    for b in range(B):
        q_last = (hs[b, T - 1] @ qW.T).reshape(1, H, D)
        ql = rope_rows(q_last, np.array([float(T - 1)]))[0]  # [H, D]
        Kt = (hs[b, EVICT:] @ kW.T).reshape(T - EVICT, H, D)
        Kt = rope_rows(Kt, np.arange(EVICT, T, dtype=np.float64))
        s = np.einsum('khd,hd->hk', Kt, ql) * SCALE  # [H, 513]
        a = np.exp(s)
        for h in range(H):
            rs = max(float(rowsum_last[b, h]), 1e-30)
            avg_w = max(float(np.mean(a[h, 1:])) / rs, 1e-6)
            prob = float(np.clip((a[h, 0] / rs) / avg_w, 0.0, 1.0))
            m[b, h] = 1.0 if U_CONST[b, h] < prob else 0.0
    return m


def _epilogue(out, results, hidden_states, q_w, k_w, v_w, o_w):
    """Add the CaM rank-1 correction per (b, h) on host."""
    rowsum_last = np.zeros((B, H), np.float64)
    for core in range(NCORES):
        r = results[core]
        for p in range(4):
            b, hl = p // 2, p % 2
            h = core * HL + hl
            rowsum_last[b, h] = float(r["sums"][p][0][T - 1])
    m_tbl = _host_m(hidden_states, q_w, k_w, rowsum_last)
    for core in range(NCORES):
        r = results[core]
        for p in range(4):
            b, hl = p // 2, p % 2
            h = core * HL + hl
            if m_tbl[b, h] == 0.0:
                continue
            rowsum = np.asarray(r["sums"][p][0], np.float64)
            tails = np.asarray(r["sums"][p][1], np.float64)
            # exact v_e from fp32 inputs
            v_row = hidden_states[b, EVICT, :] @ v_w[h * D:(h + 1) * D, :].T
            v_e = v_row * (1.0 / RB)  # [D]
            w_e = o_w[:, h * D:(h + 1) * D] @ v_e  # [HID]
            s_tail = (tails / np.maximum(rowsum, 1e-30)).astype(np.float32)
            out[b] += np.outer(s_tail, w_e).astype(np.float32)
    return out


def kernel(hidden_states, attention_mask, q_w, k_w, v_w, o_w):
    from concourse.bass_utils import run_bass_kernel_spmd

    nc = _get_nc()
    in_maps = _host_inputs(hidden_states, q_w, k_w, v_w, o_w)
    trace = bool(int(os.environ.get("BK_TRACE", "0")))
    res = run_bass_kernel_spmd(
        nc, in_maps, core_ids=list(range(NCORES)), trace=trace,
    )
    if trace and res.exec_time_ns is not None:
        print(f"HW exec time: {res.exec_time_ns} ns")
        _NC_CACHE["last_exec_ns"] = res.exec_time_ns
        _NC_CACHE["last_trace"] = res.instructions_and_trace
    results = res.results

    acc = np.zeros((HID, BT), dtype=np.float32)
    for core in range(NCORES):
        acc += np.asarray(results[core]["outT"], np.float32)
    out = np.ascontiguousarray(acc.T).reshape(B, T, HID)
    out = _epilogue(out, results, hidden_states, q_w, k_w, v_w, o_w)
    return out.astype(np.float32)
